# revision 3
# baseline (speedup 1.0000x reference)
"""HQQ int4 weight-only quantized linear for TRN2, 8-core tensor-parallel.

out[M, N] = x[M, K] @ dequant(W_q[N, K]).T
  dequant: w[n, k] = (q[n, k] - 8) * scales[n, k//128] + zeros[n, k//128]

Sharding: column-parallel over N (out_features) across 8 NeuronCores;
x replicated; outputs concatenated on host. No collectives.

v3: weights are dequantized on the host (fp32) then re-quantized to
int8 with a per-output-channel scale sw[n] = max_k|w[n,k]|/127. The
device DMAs the int8 weights (half the bytes of bf16 -> no early-window
HBM starvation), and the otherwise-idle Vector engine upcasts each
k-group tile with a single fused multiply against a broadcast f32 scale
row: wd[k, n] = wi8[k, n] * sw_bc[n]  (bf16 out). Matmul core:
  - per m-subtile of 128 rows: 3 PSUM banks (512/512/352 cols),
    accumulated over the 32 k-tiles, evicted per-bank to SBUF + HBM
  - PE warmup burst (dummy matmuls on zeros) during the initial DMA wait
    so real matmuls run at the warm 2.4 GHz clock from the start
  - panel-0 x DMA split into 4 chunks so the first matmul starts early
  - last m-subtile runs j-major so PSUM banks close staggered and the
    final eviction+DMA tail is one 352-col slice, shipped on the idle
    scalar queue
"""

import os
import sys

import numpy as np
import ml_dtypes

M = 4096
K = 4096
N = 11008
GROUP = 128
N_CORES = 8
N_SHARD = N // N_CORES  # 1376
NG = K // GROUP  # 32 quant groups == 32 k-tiles of 128
M_PANEL = 256
BF16 = ml_dtypes.bfloat16


def _install_axon_hooks_shim():
    """antenv.axon_hooks is missing from this image; run_bass_kernel_spmd
    imports it when tracing is requested (e.g. BASS_TRACE=1). Provide the
    same ctypes-based hook trn_boot would have registered."""
    import types

    try:
        import antenv.axon_hooks  # noqa: F401

        return
    except ImportError:
        pass
    try:
        import antenv
        from trn_agent_boot.trn_boot import _ntff_profile_via_ctypes

        hook = _ntff_profile_via_ctypes("/opt/axon/libaxon_pjrt.so")
        mod = types.ModuleType("antenv.axon_hooks")
        mod._hook = hook
        mod.get_axon_ntff_profile_hook = lambda: mod._hook

        def _set(h):
            mod._hook = h

        mod.set_axon_ntff_profile_hook = _set
        sys.modules["antenv.axon_hooks"] = mod
        antenv.axon_hooks = mod
    except Exception:
        pass


def build_bass(m=M, k=K, n_shard=N_SHARD, ng=NG, compile=True):
    import concourse.mybir as mybir
    import concourse.tile as tile
    from concourse import bacc

    P = 128
    MP = M_PANEL
    assert k == ng * GROUP and m % MP == 0
    f32 = mybir.dt.float32
    bf16 = mybir.dt.bfloat16
    i8 = mybir.dt.int8
    n_panels = m // MP
    nsub = MP // P  # m-subtiles per panel (2)

    nc = bacc.Bacc("TRN2", target_bir_lowering=False, debug=False)
    xT4 = nc.dram_tensor("xT4", [n_panels, P, ng, MP], bf16, kind="ExternalInput")
    wi8 = nc.dram_tensor("wi8", [k, n_shard], i8, kind="ExternalInput")
    sw = nc.dram_tensor("sw", [1, n_shard], f32, kind="ExternalInput")
    out = nc.dram_tensor("out", [m, n_shard], bf16, kind="ExternalOutput")

    n_tiles = []
    st = 0
    while st < n_shard:
        nf = min(512, n_shard - st)
        n_tiles.append((st, nf))
        st += nf

    with tile.TileContext(nc) as tc:
        with (
            tc.tile_pool(name="wdeq", bufs=ng) as wdeq_pool,
            tc.tile_pool(name="wst", bufs=6) as wst_pool,
            tc.tile_pool(name="small", bufs=1) as small_pool,
            tc.tile_pool(name="xp", bufs=3) as xp_pool,
            tc.tile_pool(name="osb", bufs=2) as osb_pool,
            tc.tile_pool(name="psum", bufs=6, space="PSUM") as psum_pool,
            tc.tile_pool(name="pwarm", bufs=1, space="PSUM") as pwarm_pool,
        ):
            # ---- PE warmup: ~3.4us of dummy matmuls flips the HAM clock
            # gate to 8/8 while the first DMAs are still in flight ----
            wz = small_pool.tile([P, 512], bf16, tag="wz")
            nc.vector.memset(wz[:], 0.0)
            pw = pwarm_pool.tile([P, 512], f32, tag="pw")
            for _ in range(8):
                nc.tensor.matmul(pw, wz[:, :P], wz[:], start=True, stop=True)

            # ---- per-channel weight scale, broadcast to 128 partitions
            # (SWDGE queue, separate from the weight/x streams) ----
            sw_bc = small_pool.tile([P, n_shard], f32, tag="swbc")
            nc.gpsimd.dma_start(sw_bc[:], sw[0:1, :].to_broadcast((P, n_shard)))

            # ---- int8 weight tiles in (sync queue), upcast+scale on DVE ----
            wdeq_tiles = []
            for g in range(ng):
                wst = wst_pool.tile([P, n_shard], i8, tag="wst")
                nc.sync.dma_start(wst[:], wi8[g * P : (g + 1) * P, :])
                wd = wdeq_pool.tile([P, n_shard], bf16, tag="wdeq")
                nc.vector.tensor_mul(wd[:], wst[:], sw_bc[:])
                wdeq_tiles.append(wd)

            # ---- panel-0 x in 4 chunks (scalar queue) so chunk 0 lands
            # fast and the first matmul isn't gated on a 2MB transfer ----
            xp_tiles = {}
            xp_tiles[0] = xp_pool.tile([P, ng, MP], bf16, tag="xp", name="xp0")
            gchunk = ng // 4
            for c in range(4):
                sl = slice(c * gchunk, (c + 1) * gchunk)
                nc.scalar.dma_start(xp_tiles[0][:, sl, :], xT4[0][:, sl, :])

            def evict(psums, ms_abs, dma_ring=nc.sync):
                osb = osb_pool.tile([P, n_shard], bf16, tag="osb")
                m0 = ms_abs * P
                for j, (st, nf) in enumerate(n_tiles):
                    nc.any.tensor_copy(osb[:, st : st + nf], psums[j])
                    dma_ring.dma_start(
                        out[m0 : m0 + P, st : st + nf], osb[:, st : st + nf]
                    )

            def emit_panel_k_outer(xp, mp):
                # both m-subtiles' k-sweeps interleaved: 6 open psum banks;
                # halves the weight-DMA rate needed while weights stream in.
                pss = []
                for ms in range(nsub):
                    row = []
                    for j, (st, nf) in enumerate(n_tiles):
                        ps = psum_pool.tile([P, 512], f32, tag="ps", name="psA")[:, :nf]
                        row.append(ps)
                    pss.append(row)
                for g in range(ng):
                    for ms in range(nsub):
                        lhsT = xp[:, g, ms * P : (ms + 1) * P]
                        for j, (st, nf) in enumerate(n_tiles):
                            nc.tensor.matmul(
                                pss[ms][j],
                                lhsT,
                                wdeq_tiles[g][:, st : st + nf],
                                start=(g == 0),
                                stop=(g == ng - 1),
                            )
                for ms in range(nsub):
                    evict(pss[ms], mp * nsub + ms)

            def emit_panel_ms_inner(xp, mp):
                for ms in range(nsub):
                    psums = []
                    for j, (st, nf) in enumerate(n_tiles):
                        ps = psum_pool.tile([P, 512], f32, tag="ps", name="psB")[:, :nf]
                        psums.append(ps)
                    for g in range(ng):
                        lhsT = xp[:, g, ms * P : (ms + 1) * P]
                        for j, (st, nf) in enumerate(n_tiles):
                            nc.tensor.matmul(
                                psums[j],
                                lhsT,
                                wdeq_tiles[g][:, st : st + nf],
                                start=(g == 0),
                                stop=(g == ng - 1),
                            )
                    evict(psums, mp * nsub + ms)

            def emit_panel_last(xp, mp):
                # ms0 as usual; ms1 j-major so the 3 banks close staggered
                # and the end-of-kernel tail is a single 352-col slice.
                ms = 0
                psums = []
                for j, (st, nf) in enumerate(n_tiles):
                    ps = psum_pool.tile([P, 512], f32, tag="ps", name="psB")[:, :nf]
                    psums.append(ps)
                for g in range(ng):
                    lhsT = xp[:, g, :P]
                    for j, (st, nf) in enumerate(n_tiles):
                        nc.tensor.matmul(
                            psums[j],
                            lhsT,
                            wdeq_tiles[g][:, st : st + nf],
                            start=(g == 0),
                            stop=(g == ng - 1),
                        )
                evict(psums, mp * nsub)

                ms_abs = mp * nsub + 1
                m0 = ms_abs * P
                osb = osb_pool.tile([P, n_shard], bf16, tag="osb")
                for j, (st, nf) in enumerate(n_tiles):
                    ps = psum_pool.tile([P, 512], f32, tag="ps", name="psC")[:, :nf]
                    for g in range(ng):
                        nc.tensor.matmul(
                            ps,
                            xp[:, g, P : 2 * P],
                            wdeq_tiles[g][:, st : st + nf],
                            start=(g == 0),
                            stop=(g == ng - 1),
                        )
                    nc.any.tensor_copy(osb[:, st : st + nf], ps)
                    nc.scalar.dma_start(
                        out[m0 : m0 + P, st : st + nf], osb[:, st : st + nf]
                    )

            for mp in range(n_panels):
                # keep 2 panels of x prefetch in flight
                for q in (mp + 1, mp + 2):
                    if q < n_panels and q not in xp_tiles:
                        xp_tiles[q] = xp_pool.tile(
                            [P, ng, MP], bf16, tag="xp", name=f"xp{q}"
                        )
                        nc.scalar.dma_start(xp_tiles[q][:], xT4[q])
                if mp < 2:
                    emit_panel_k_outer(xp_tiles[mp], mp)
                elif mp < n_panels - 1:
                    emit_panel_ms_inner(xp_tiles[mp], mp)
                else:
                    emit_panel_last(xp_tiles[mp], mp)

    if compile:
        nc.compile()
    return nc


def host_prep(x, W_q, scales, zeros):
    """Host-side prep: x tiled for the kernel layout; weights dequantized
    in fp32 then requantized to int8 with a per-output-channel scale."""
    x = np.asarray(x)
    n_panels = M // M_PANEL
    # x tiled: [panel, k_in_group, group, m_in_panel]
    xT4 = np.ascontiguousarray(
        x.reshape(n_panels, M_PANEL, NG, GROUP).transpose(0, 3, 2, 1)
    )
    q = np.asarray(W_q).astype(np.float32).reshape(N, NG, GROUP)
    s = np.asarray(scales).astype(np.float32)[:, :, None]
    z = np.asarray(zeros).astype(np.float32)[:, :, None]
    w = ((q - 8.0) * s + z).reshape(N, K)  # [N, K] fp32
    amax = np.maximum(np.abs(w).max(axis=1), 1e-8)
    sw = (amax / 127.0).astype(np.float32)  # [N]
    wi = np.clip(np.rint(w / sw[:, None]), -127, 127).astype(np.int8)
    wiT = np.ascontiguousarray(wi.T)  # [K, N] int8
    return xT4, wiT, sw


_NC_CACHE = {}
_LAST_IN_MAPS = None


def kernel(x, W_q, scales, zeros):
    _install_axon_hooks_shim()
    from concourse.bass_utils import run_bass_kernel_spmd

    xT4, wiT, sw = host_prep(x, W_q, scales, zeros)

    if "nc" not in _NC_CACHE:
        _NC_CACHE["nc"] = build_bass()
    nc = _NC_CACHE["nc"]

    in_maps = []
    for c in range(N_CORES):
        lo, hi = c * N_SHARD, (c + 1) * N_SHARD
        in_maps.append(
            {
                "xT4": xT4,
                "wi8": np.ascontiguousarray(wiT[:, lo:hi]),
                "sw": np.ascontiguousarray(sw[None, lo:hi]),
            }
        )

    global _LAST_IN_MAPS
    _LAST_IN_MAPS = in_maps
    res = run_bass_kernel_spmd(nc, in_maps, list(range(N_CORES)))
    out = np.concatenate([res.results[c]["out"] for c in range(N_CORES)], axis=1)
    return out.astype(BF16, copy=False)


# revision 4
# speedup vs baseline: 1.0225x; 1.0225x over previous
"""HQQ int4 weight-only quantized linear for TRN2, 8-core tensor-parallel.

out[M, N] = x[M, K] @ dequant(W_q[N, K]).T
  dequant: w[n, k] = (q[n, k] - 8) * scales[n, k//128] + zeros[n, k//128]

Sharding: column-parallel over N (out_features) across 8 NeuronCores;
x replicated; outputs concatenated on host. No collectives.

v4: weights are fully dequantized on the host (fp32 math, bf16 result)
and shipped as wT[K, n_shard] per core, so the device does matmul only.
The early window is HBM-bandwidth-critical (weights 11.3MB + first x
panels must land before the PE catches up), so ALL input loads go on a
single sync-engine DMA queue in just-in-time interleaved order (weight
tiles paced with x chunks); queue FIFO = strict priority. Outputs go on
the scalar queue. Also:
  - PE warmup burst (dummy matmuls on zeros) during the initial DMA wait
    so real matmuls run at the warm 2.4 GHz clock from the start
  - per m-subtile of 128 rows: 3 PSUM banks (512/512/352 cols),
    accumulated over the 32 k-tiles, evicted per-bank to SBUF + HBM
  - last m-subtile runs j-major so PSUM banks close staggered and the
    final eviction+DMA tail is one 352-col slice
"""

import os
import sys

import numpy as np
import ml_dtypes

M = 4096
K = 4096
N = 11008
GROUP = 128
N_CORES = 8
N_SHARD = N // N_CORES  # 1376
NG = K // GROUP  # 32 quant groups == 32 k-tiles of 128
M_PANEL = 256
BF16 = ml_dtypes.bfloat16


def _install_axon_hooks_shim():
    """antenv.axon_hooks is missing from this image; run_bass_kernel_spmd
    imports it when tracing is requested (e.g. BASS_TRACE=1). Provide the
    same ctypes-based hook trn_boot would have registered."""
    import types

    try:
        import antenv.axon_hooks  # noqa: F401

        return
    except ImportError:
        pass
    try:
        import antenv
        from trn_agent_boot.trn_boot import _ntff_profile_via_ctypes

        hook = _ntff_profile_via_ctypes("/opt/axon/libaxon_pjrt.so")
        mod = types.ModuleType("antenv.axon_hooks")
        mod._hook = hook
        mod.get_axon_ntff_profile_hook = lambda: mod._hook

        def _set(h):
            mod._hook = h

        mod.set_axon_ntff_profile_hook = _set
        sys.modules["antenv.axon_hooks"] = mod
        antenv.axon_hooks = mod
    except Exception:
        pass


def build_bass(m=M, k=K, n_shard=N_SHARD, ng=NG, compile=True):
    import concourse.mybir as mybir
    import concourse.tile as tile
    from concourse import bacc

    P = 128
    MP = M_PANEL
    assert k == ng * GROUP and m % MP == 0
    f32 = mybir.dt.float32
    bf16 = mybir.dt.bfloat16
    n_panels = m // MP
    nsub = MP // P  # m-subtiles per panel (2)

    nc = bacc.Bacc("TRN2", target_bir_lowering=False, debug=False)
    xT4 = nc.dram_tensor("xT4", [n_panels, P, ng, MP], bf16, kind="ExternalInput")
    wT = nc.dram_tensor("wT", [k, n_shard], bf16, kind="ExternalInput")
    out = nc.dram_tensor("out", [m, n_shard], bf16, kind="ExternalOutput")

    n_tiles = []
    st = 0
    while st < n_shard:
        nf = min(512, n_shard - st)
        n_tiles.append((st, nf))
        st += nf

    with tile.TileContext(nc) as tc:
        with (
            tc.tile_pool(name="wdeq", bufs=ng) as wdeq_pool,
            tc.tile_pool(name="small", bufs=1) as small_pool,
            tc.tile_pool(name="xp", bufs=3) as xp_pool,
            tc.tile_pool(name="osb", bufs=2) as osb_pool,
            tc.tile_pool(name="psum", bufs=6, space="PSUM") as psum_pool,
            tc.tile_pool(name="pwarm", bufs=1, space="PSUM") as pwarm_pool,
        ):
            # ---- PE warmup: ~3.4us of dummy matmuls flips the HAM clock
            # gate to 8/8 while the first DMAs are still in flight ----
            wz = small_pool.tile([P, 512], bf16, tag="wz")
            nc.vector.memset(wz[:], 0.0)
            pw = pwarm_pool.tile([P, 512], f32, tag="pw")
            for _ in range(8):
                nc.tensor.matmul(pw, wz[:, :P], wz[:], start=True, stop=True)

            # ---- all input loads on ONE sync-queue FIFO, just-in-time
            # interleaved: w tiles paced against the panel-0 x chunks so
            # neither stream starves the other in the HBM-critical window.
            # Queue order: w0 w1 | xc0 | w2..w9 | xc1 | w10..w17 | xc2 |
            #              w18..w25 | xc3 | w26..w31 | xp1 xp2 ----
            wdeq_tiles = [None] * ng
            xp_tiles = {}
            xp_tiles[0] = xp_pool.tile([P, ng, MP], bf16, tag="xp", name="xp0")
            gchunk = ng // 4

            def load_w(g):
                wd = wdeq_pool.tile([P, n_shard], bf16, tag="wdeq")
                nc.sync.dma_start(wd[:], wT[g * P : (g + 1) * P, :])
                wdeq_tiles[g] = wd

            def load_xchunk(c):
                sl = slice(c * gchunk, (c + 1) * gchunk)
                nc.sync.dma_start(xp_tiles[0][:, sl, :], xT4[0][:, sl, :])

            load_w(0)
            load_w(1)
            load_xchunk(0)
            for g in range(2, 10):
                load_w(g)
            load_xchunk(1)
            for g in range(10, 18):
                load_w(g)
            load_xchunk(2)
            for g in range(18, 26):
                load_w(g)
            load_xchunk(3)
            for g in range(26, ng):
                load_w(g)

            def evict(psums, ms_abs):
                osb = osb_pool.tile([P, n_shard], bf16, tag="osb")
                m0 = ms_abs * P
                for j, (st, nf) in enumerate(n_tiles):
                    nc.any.tensor_copy(osb[:, st : st + nf], psums[j])
                    nc.scalar.dma_start(
                        out[m0 : m0 + P, st : st + nf], osb[:, st : st + nf]
                    )

            def emit_panel_k_outer(xp, mp):
                # both m-subtiles' k-sweeps interleaved: 6 open psum banks;
                # halves the weight-DMA rate needed while weights stream in.
                pss = []
                for ms in range(nsub):
                    row = []
                    for j, (st, nf) in enumerate(n_tiles):
                        ps = psum_pool.tile([P, 512], f32, tag="ps", name="psA")[:, :nf]
                        row.append(ps)
                    pss.append(row)
                for g in range(ng):
                    for ms in range(nsub):
                        lhsT = xp[:, g, ms * P : (ms + 1) * P]
                        for j, (st, nf) in enumerate(n_tiles):
                            nc.tensor.matmul(
                                pss[ms][j],
                                lhsT,
                                wdeq_tiles[g][:, st : st + nf],
                                start=(g == 0),
                                stop=(g == ng - 1),
                            )
                for ms in range(nsub):
                    evict(pss[ms], mp * nsub + ms)

            def emit_panel_ms_inner(xp, mp):
                for ms in range(nsub):
                    psums = []
                    for j, (st, nf) in enumerate(n_tiles):
                        ps = psum_pool.tile([P, 512], f32, tag="ps", name="psB")[:, :nf]
                        psums.append(ps)
                    for g in range(ng):
                        lhsT = xp[:, g, ms * P : (ms + 1) * P]
                        for j, (st, nf) in enumerate(n_tiles):
                            nc.tensor.matmul(
                                psums[j],
                                lhsT,
                                wdeq_tiles[g][:, st : st + nf],
                                start=(g == 0),
                                stop=(g == ng - 1),
                            )
                    evict(psums, mp * nsub + ms)

            def emit_panel_last(xp, mp):
                # ms0 as usual; ms1 j-major so the 3 banks close staggered
                # and the end-of-kernel tail is a single 352-col slice.
                psums = []
                for j, (st, nf) in enumerate(n_tiles):
                    ps = psum_pool.tile([P, 512], f32, tag="ps", name="psB")[:, :nf]
                    psums.append(ps)
                for g in range(ng):
                    lhsT = xp[:, g, :P]
                    for j, (st, nf) in enumerate(n_tiles):
                        nc.tensor.matmul(
                            psums[j],
                            lhsT,
                            wdeq_tiles[g][:, st : st + nf],
                            start=(g == 0),
                            stop=(g == ng - 1),
                        )
                evict(psums, mp * nsub)

                ms_abs = mp * nsub + 1
                m0 = ms_abs * P
                osb = osb_pool.tile([P, n_shard], bf16, tag="osb")
                for j, (st, nf) in enumerate(n_tiles):
                    ps = psum_pool.tile([P, 512], f32, tag="ps", name="psC")[:, :nf]
                    for g in range(ng):
                        nc.tensor.matmul(
                            ps,
                            xp[:, g, P : 2 * P],
                            wdeq_tiles[g][:, st : st + nf],
                            start=(g == 0),
                            stop=(g == ng - 1),
                        )
                    nc.any.tensor_copy(osb[:, st : st + nf], ps)
                    nc.scalar.dma_start(
                        out[m0 : m0 + P, st : st + nf], osb[:, st : st + nf]
                    )

            for mp in range(n_panels):
                # keep 2 panels of x prefetch in flight (same sync queue,
                # behind the weight stream)
                for q in (mp + 1, mp + 2):
                    if q < n_panels and q not in xp_tiles:
                        xp_tiles[q] = xp_pool.tile(
                            [P, ng, MP], bf16, tag="xp", name=f"xp{q}"
                        )
                        nc.sync.dma_start(xp_tiles[q][:], xT4[q])
                if mp < 2:
                    emit_panel_k_outer(xp_tiles[mp], mp)
                elif mp < n_panels - 1:
                    emit_panel_ms_inner(xp_tiles[mp], mp)
                else:
                    emit_panel_last(xp_tiles[mp], mp)

    if compile:
        nc.compile()
    return nc


def host_prep(x, W_q, scales, zeros):
    """Host-side prep: x tiled for the kernel layout; weights fully
    dequantized in fp32 and transposed to [K, N] bf16."""
    x = np.asarray(x)
    n_panels = M // M_PANEL
    # x tiled: [panel, k_in_group, group, m_in_panel]
    xT4 = np.ascontiguousarray(
        x.reshape(n_panels, M_PANEL, NG, GROUP).transpose(0, 3, 2, 1)
    )
    q = np.asarray(W_q).astype(np.float32).reshape(N, NG, GROUP)
    s = np.asarray(scales).astype(np.float32)[:, :, None]
    z = np.asarray(zeros).astype(np.float32)[:, :, None]
    w = ((q - 8.0) * s + z).astype(BF16).reshape(N, K)  # [N, K]
    wT_full = np.ascontiguousarray(w.T)  # [K, N]
    return xT4, wT_full


_NC_CACHE = {}
_LAST_IN_MAPS = None


def kernel(x, W_q, scales, zeros):
    _install_axon_hooks_shim()
    from concourse.bass_utils import run_bass_kernel_spmd

    xT4, wT_full = host_prep(x, W_q, scales, zeros)

    if "nc" not in _NC_CACHE:
        _NC_CACHE["nc"] = build_bass()
    nc = _NC_CACHE["nc"]

    in_maps = []
    for c in range(N_CORES):
        lo, hi = c * N_SHARD, (c + 1) * N_SHARD
        in_maps.append(
            {
                "xT4": xT4,
                "wT": np.ascontiguousarray(wT_full[:, lo:hi]),
            }
        )

    global _LAST_IN_MAPS
    _LAST_IN_MAPS = in_maps
    res = run_bass_kernel_spmd(nc, in_maps, list(range(N_CORES)))
    out = np.concatenate([res.results[c]["out"] for c in range(N_CORES)], axis=1)
    return out.astype(BF16, copy=False)
